# revision 4
# baseline (speedup 1.0000x reference)
"""Greedy attention-LAP kernel for TRN2 (8 NeuronCores, data-parallel over batch).

Algorithm per batch b (n1=n2=512):
  mask = cols < ncols[b]
  for r in 0..511:
    logits = where(mask, s[b,r,:], -1e30); p = softmax(logits)
    out[b,r,:] = p if r < nrows[b] else 0
    if r < nrows[b]: mask[argmax(logits)] = False

Device kernel per core (16 batches), two phases over 64 blocks of 8 rows:
  Phase 1 (sequential): mask kept as removed-step code q_enc in PSUM,
    updated by PE matmul accumulation of per-block scatter deltas; per block
    extract top-8 candidates per row (max8/max_index), shuffle to batch
    layout via selector matmuls, resolve the 8 rows sequentially.
  Phase 2 (pipelined): reconstruct per-row mask from q_enc, compute
    e' = exp(s - mask - rowmax) with accumulated row sum, emit
    q = round(255*e') as uint8 plus the row sums in f32.

Wall-clock is dominated by the axon tunnel (~80 MB/s, half-duplex), so the
host side is built around it: the only large upload is s itself (one
device_put, pass-through, no host copies); every input-independent table is
an inline NEFF constant and the nrows/ncols-derived tables are computed
on-device from a [128,2] f32 upload. The output comes back as uint8 q +
f32 row sums (34 MB instead of 134 MB) and is dequantized on the host as
out = q * (r < nrows) / (255 * sum), overlapping per-shard downloads with
dequantization.
"""

import os
import sys
from concurrent.futures import ThreadPoolExecutor

import numpy as np

sys.path.insert(0, "/opt/trn_rl_repo")
sys.path.insert(0, "/opt/trn_rl_repo/concourse")

B, N1, N2 = 128, 512, 512
NCORES = 8
BL = 16  # batches per core
NBLK = 64  # blocks of 8 rows
RPB = 8  # rows per block

QNEVER = 2048.0  # q_enc never-removed offset:  q_enc = 2048 - r
BIGP = float(2.0**101)  # mask scale; relu(a*x) = a*relu(x)
EXPB = -12.0  # fixed softmax shift for the f32 fallback output

_nc_cache = {}


def build_nc():
    import concourse.bass as bass
    import concourse.bacc as bacc
    import concourse.tile as tile
    from concourse import mybir

    f32 = mybir.dt.float32
    f16 = mybir.dt.float16
    u8 = mybir.dt.uint8
    u32 = mybir.dt.uint32
    Alu = mybir.AluOpType
    Act = mybir.ActivationFunctionType

    nc = bacc.Bacc(None, target_bir_lowering=False)

    s_in = nc.dram_tensor("s", [BL, N1, N2], f32, kind="ExternalInput")
    nrc_in = nc.dram_tensor("nrc", [BL, 2], f32, kind="ExternalInput")
    out_dram = nc.dram_tensor("out", [BL, N1, N2], f32, kind="ExternalOutput")
    qout_dram = nc.dram_tensor("qout", [BL, N1, N2], u8, kind="ExternalOutput")
    sums_dram = nc.dram_tensor("sums", [BL, N1], f32, kind="ExternalOutput")

    # ---- inline constant tables (baked into the NEFF, no upload) ----
    c = np.arange(N2, dtype=np.float32)
    iota_np = np.broadcast_to(c[None, :], (BL, N2))
    qnminus_np = QNEVER - iota_np
    iota16_np = iota_np.astype(np.float16)
    w8_np = np.broadcast_to(
        np.tile(np.arange(8, 0, -1, dtype=np.float32), 8)[None, :], (BL, 64)
    )
    rep16_np = np.zeros((BL, 128), dtype=np.float16)
    for b in range(BL):
        rep16_np[b, b::BL] = 1.0
    selpack_np = np.zeros((128, RPB, BL), dtype=np.float16)
    for j in range(RPB):
        for b in range(BL):
            selpack_np[BL * j + b, j, b] = 1.0
    Ks = np.arange(NBLK)
    p = np.arange(128)
    biasP1_np = np.broadcast_to(
        (RPB * Ks - QNEVER)[None, :].astype(np.float64), (128, NBLK)
    ).astype(np.float32)
    rowp = RPB * Ks[None, :] + (p // BL)[:, None]  # [128, NBLK] row index
    biasR2_np = ((rowp - QNEVER) * BIGP).astype(np.float32)
    rowp_np = rowp.astype(np.float32)

    iota_d = nc.inline_tensor(np.ascontiguousarray(iota_np), "iota_c")
    qnminus_d = nc.inline_tensor(np.ascontiguousarray(qnminus_np), "qnminus_c")
    iota16_d = nc.inline_tensor(np.ascontiguousarray(iota16_np), "iota16_c")
    w8_d = nc.inline_tensor(np.ascontiguousarray(w8_np), "w8_c")
    rep16_d = nc.inline_tensor(rep16_np, "rep16_c")
    selpack_d = nc.inline_tensor(selpack_np, "selpack_c")
    biasP1_d = nc.inline_tensor(biasP1_np, "biasP1_c")
    biasR2_d = nc.inline_tensor(biasR2_np, "biasR2_c")
    rowp_d = nc.inline_tensor(np.ascontiguousarray(rowp_np), "rowp_c")

    # phase-1/2 layout: partition p = j*16 + b  (j = row in block, b = batch)
    # manual APs: for block K, partition (j,b) maps to dram row s[b, 8K+j, :]
    def blk_ap(dram_t, K):
        a = dram_t[:]
        return bass.AP(
            tensor=a.tensor,
            offset=a.offset + K * RPB * N2,
            ap=[[N2, RPB], [N1 * N2, BL], [1, N2]],
        )

    def sums_ap(K):
        a = sums_dram[:]
        return bass.AP(
            tensor=a.tensor,
            offset=a.offset + K * RPB,
            ap=[[1, RPB], [N1, BL], [1, 1]],
        )

    s_r = [blk_ap(s_in, K) for K in range(NBLK)]
    out_r = [blk_ap(out_dram, K) for K in range(NBLK)]
    q_r = [blk_ap(qout_dram, K) for K in range(NBLK)]

    with tile.TileContext(nc) as tc:
        import contextlib

        ctx = contextlib.ExitStack()
        with ctx:
            consts = ctx.enter_context(tc.tile_pool(name="consts", bufs=1))
            s_pool = ctx.enter_context(tc.tile_pool(name="s_pool", bufs=1))
            big = ctx.enter_context(tc.tile_pool(name="big", bufs=3))
            big2 = ctx.enter_context(tc.tile_pool(name="big2", bufs=5))
            outp_pool = ctx.enter_context(tc.tile_pool(name="outp", bufs=3))
            qp_pool = ctx.enter_context(tc.tile_pool(name="qp", bufs=3))
            small = ctx.enter_context(tc.tile_pool(name="small", bufs=4))
            psum_q = ctx.enter_context(tc.tile_pool(name="psq", bufs=1, space="PSUM"))
            psum_c = ctx.enter_context(tc.tile_pool(name="psc", bufs=2, space="PSUM"))

            # ---- load inline consts + nrc into SBUF ----
            def load_const(dram, shape, dtype, tag):
                t = consts.tile(shape, dtype, tag=tag)
                nc.sync.dma_start(out=t, in_=dram[:])
                return t

            c_nrc = load_const(nrc_in, [BL, 2], f32, "c_nrc")
            c_iotaf = load_const(iota_d, [BL, N2], f32, "c_iotaf")
            c_qnminus = load_const(qnminus_d, [BL, N2], f32, "c_qnminus")
            c_iota = load_const(iota16_d, [BL, N2], f16, "c_iota")
            c_w8 = load_const(w8_d, [BL, 64], f32, "c_w8")
            c_rep16 = load_const(rep16_d, [BL, 128], f16, "c_rep16")
            c_sel = load_const(selpack_d, [128, RPB, BL], f16, "c_sel")
            c_biasP1 = load_const(biasP1_d, [128, NBLK], f32, "c_biasP1")
            c_biasR2 = load_const(biasR2_d, [128, NBLK], f32, "c_biasR2")
            c_rowp = load_const(rowp_d, [128, NBLK], f32, "c_rowp")

            # ---- derive nrows/ncols-dependent tables on device ----
            # rstep[b,r] = (r < nrows[b]) * (2048 - r)
            c_rstep = consts.tile([BL, N1], f32, tag="c_rstep")
            nc.vector.scalar_tensor_tensor(
                out=c_rstep, in0=c_iotaf[:], scalar=c_nrc[:, 0:1],
                in1=c_qnminus[:], op0=Alu.is_lt, op1=Alu.mult,
            )
            # qinit[b,c] = (c >= ncols[b]) * 2050  (f16)
            c_qinit = consts.tile([BL, N2], f16, tag="c_qinit")
            nc.vector.tensor_scalar(
                out=c_qinit, in0=c_iotaf[:], scalar1=c_nrc[:, 1:2],
                scalar2=QNEVER + 2.0, op0=Alu.is_ge, op1=Alu.mult,
            )
            # nrep[p] = nrows[p % 16] replicated to the (j,b) layout
            c_nrh = consts.tile([BL, 1], f16, tag="c_nrh")
            nc.vector.tensor_copy(c_nrh, c_nrc[:, 0:1])
            nrep_ps = psum_c.tile([128, 1], f32, tag="nrep_ps")
            nc.tensor.matmul(
                nrep_ps[:], c_rep16[:], c_nrh[:], start=True, stop=True,
                skip_group_check=True,
            )
            c_nrep = consts.tile([128, 1], f32, tag="c_nrep")
            nc.vector.tensor_copy(c_nrep, nrep_ps[:])
            # actflag[p,K] = (rowp < nrows[b])
            c_actflag = consts.tile([128, NBLK], f32, tag="c_actflag")
            nc.vector.tensor_scalar(
                out=c_actflag, in0=c_rowp[:], scalar1=c_nrep[:],
                scalar2=None, op0=Alu.is_lt,
            )

            # ---- load s fully resident ----
            s_tiles = []
            for K in range(NBLK):
                st = s_pool.tile([128, N2], f32, tag=f"s{K}")
                nc.sync.dma_start(out=st, in_=s_r[K])
                s_tiles.append(st)

            bias_exp = consts.tile([128, 1], f32)
            nc.vector.memset(bias_exp, EXPB)

            # ---- q_enc PSUM accumulator init ----
            qenc = psum_q.tile([128, N2], f32)
            nc.tensor.matmul(
                qenc[:], c_rep16[:], c_qinit[:], start=True, stop=True,
                skip_group_check=True,
            )

            val8_tiles = [None] * NBLK

            def phase2(K):
                tp2 = big2.tile([128, N2], f32, tag="tp2")
                nc.scalar.activation(
                    tp2, qenc[:], Act.Relu,
                    bias=c_biasR2[:, K : K + 1], scale=BIGP,
                )
                x2 = big2.tile([128, N2], f32, tag="x2")
                nc.gpsimd.tensor_tensor(
                    out=x2, in0=s_tiles[K][:], in1=tp2[:], op=Alu.subtract
                )
                # f32 fallback output branch
                e = big2.tile([128, N2], f32, tag="e")
                sumexp = small.tile([128, 1], f32, tag="sumexp")
                nc.scalar.activation(
                    e, x2[:], Act.Exp, bias=bias_exp[:], scale=1.0,
                    accum_out=sumexp,
                )
                rs = small.tile([128, 1], f32, tag="rs")
                nc.vector.reciprocal(rs, sumexp[:])
                rs2 = small.tile([128, 1], f32, tag="rs2")
                nc.vector.tensor_scalar(
                    out=rs2, in0=rs[:], scalar1=c_actflag[:, K : K + 1],
                    scalar2=None, op0=Alu.mult,
                )
                outp = outp_pool.tile([128, N2], f32, tag="outp")
                nc.scalar.mul(outp, e[:], rs2[:])
                nc.sync.dma_start(out=out_r[K], in_=outp[:])
                # u8 branch: q = round(255 * exp(x - rowmax)), sums = sum e'
                negmax = small.tile([128, 1], f32, tag="negmax")
                nc.vector.tensor_scalar(
                    out=negmax, in0=val8_tiles[K][:, 0:1], scalar1=-1.0,
                    scalar2=None, op0=Alu.mult,
                )
                e2 = big2.tile([128, N2], f32, tag="e2")
                sum2 = small.tile([128, 1], f32, tag="sum2")
                nc.scalar.activation(
                    e2, x2[:], Act.Exp, bias=negmax[:], scale=1.0,
                    accum_out=sum2,
                )
                nc.sync.dma_start(out=sums_ap(K), in_=sum2[:])
                q8 = qp_pool.tile([128, N2], u8, tag="q8")
                nc.vector.tensor_scalar(
                    out=q8, in0=e2[:], scalar1=255.0, scalar2=None,
                    op0=Alu.mult,
                )
                nc.sync.dma_start(out=q_r[K], in_=q8[:])

            PHASE2_LAG = 2

            for K in range(NBLK):
                # ---------- phase 1: extraction ----------
                # tpos = max(qenc + (8K-2048), 0): 0 = allowed, >=1 = removed
                tpos = big.tile([128, N2], f32, tag="tpos")
                nc.vector.tensor_scalar(
                    out=tpos, in0=qenc[:], scalar1=c_biasP1[:, K : K + 1],
                    scalar2=0.0, op0=Alu.add, op1=Alu.max,
                )
                x = big.tile([128, N2], f32, tag="x")
                nc.vector.scalar_tensor_tensor(
                    out=x, in0=tpos[:], scalar=-2.0e30, in1=s_tiles[K][:],
                    op0=Alu.mult, op1=Alu.add,
                )
                val8 = small.tile([128, 8], f32, tag="val8")
                nc.vector.max(val8, x[:])
                val8_tiles[K] = val8
                idx8u = small.tile([128, 8], u32, tag="idx8u")
                nc.vector.max_index(idx8u, val8[:], x[:])
                idx8h = small.tile([128, 8], f16, tag="idx8h")
                nc.vector.tensor_copy(idx8h, idx8u[:])

                # ---------- shuffle indices to batch-partition layout ----------
                cand_ps = psum_c.tile([BL, 64], f32, tag="cand")
                for j in range(RPB):
                    nc.tensor.matmul(
                        cand_ps[:, 8 * j : 8 * j + 8],
                        c_sel[:, j, :], idx8h[:],
                        start=True, stop=True, skip_group_check=True,
                    )
                cidx = small.tile([BL, 64], f32, tag="cidx")
                nc.vector.tensor_copy(cidx, cand_ps[:])

                # ---------- resolve 8 rows sequentially ----------
                # W[b, 8j+k] = (8-k) while candidate k of row j is alive, 0 after.
                # Substep 0 needs no argmax: row 0's pick is its top candidate.
                W = small.tile([BL, 64], f32, tag="W")
                picksF = small.tile([BL, RPB], f32, tag="picksF")
                m2 = small.tile([BL, 1], f32, tag="m2")
                scr = small.tile([BL, 8], f32, tag="scr")
                for j in range(RPB):
                    r = RPB * K + j
                    if j == 0:
                        pick_ap = cidx[:, 0:1]
                    else:
                        pick_ap = picksF[:, j : j + 1]
                        nc.vector.reduce_max(
                            m2, W[:, 8 * j : 8 * j + 8], axis=mybir.AxisListType.X
                        )
                        nc.vector.scalar_tensor_tensor(
                            out=scr, in0=W[:, 8 * j : 8 * j + 8],
                            scalar=m2[:], in1=cidx[:, 8 * j : 8 * j + 8],
                            op0=Alu.is_equal, op1=Alu.mult,
                            accum_out=pick_ap,
                        )
                    if j < RPB - 1:
                        lo = 8 * (j + 1)
                        w_src = c_w8 if j == 0 else W
                        nc.vector.scalar_tensor_tensor(
                            out=W[:, lo:], in0=cidx[:, lo:],
                            scalar=pick_ap, in1=w_src[:, lo:],
                            op0=Alu.not_equal, op1=Alu.mult,
                        )
                    oh512 = small.tile([BL, N2], f16, tag="oh512")
                    nc.vector.tensor_scalar(
                        out=oh512, in0=c_iota[:],
                        scalar1=pick_ap,
                        scalar2=c_rstep[:, r : r + 1],
                        op0=Alu.is_equal, op1=Alu.mult,
                    )
                    nc.tensor.matmul(
                        qenc[:], c_rep16[:], oh512[:],
                        start=False, stop=True, skip_group_check=True,
                    )

                if K >= PHASE2_LAG:
                    phase2(K - PHASE2_LAG)

            for KK in range(NBLK - PHASE2_LAG, NBLK):
                phase2(KK)

    nc.compile()
    return nc


def _get_runtime():
    if "rt" in _nc_cache:
        return _nc_cache["rt"]

    import jax
    from jax.sharding import Mesh, PartitionSpec, NamedSharding

    from jax.experimental.shard_map import shard_map

    from concourse import bass2jax, mybir
    from concourse.bass2jax import _bass_exec_p, install_neuronx_cc_hook

    install_neuronx_cc_hook()

    nc = build_nc()

    partition_name = nc.partition_id_tensor.name if nc.partition_id_tensor else None
    in_names, out_names, out_avals = [], [], []
    for alloc in nc.m.functions[0].allocations:
        if not isinstance(alloc, mybir.MemoryLocationSet):
            continue
        name = alloc.memorylocations[0].name
        if alloc.kind == "ExternalInput":
            if name != partition_name:
                in_names.append(name)
        elif alloc.kind == "ExternalOutput":
            out_avals.append(
                jax.core.ShapedArray(
                    tuple(alloc.tensor_shape), mybir.dt.np(alloc.dtype)
                )
            )
            out_names.append(name)
    in_names_full = list(in_names) + ([partition_name] if partition_name else [])

    def _body(*args):
        operands = list(args)
        if partition_name is not None:
            operands.append(bass2jax.partition_id_tensor())
        return tuple(
            _bass_exec_p.bind(
                *operands,
                out_avals=tuple(out_avals),
                in_names=tuple(in_names_full),
                out_names=tuple(out_names),
                lowering_input_output_aliases=(),
                sim_require_finite=True,
                sim_require_nnan=True,
                nc=nc,
            )
        )

    devices = jax.devices()[:NCORES]
    mesh = Mesh(np.asarray(devices), ("core",))
    sharded = jax.jit(
        shard_map(
            _body, mesh=mesh,
            in_specs=(PartitionSpec("core"),) * len(in_names),
            out_specs=(PartitionSpec("core"),) * len(out_names),
            check_rep=False,
        ),
        keep_unused=True,
    )

    rt = {
        "jax": jax,
        "nc": nc,
        "sharded": sharded,
        "sh": NamedSharding(mesh, PartitionSpec("core")),
        "in_names": in_names,
        "out_names": out_names,
        "pool": ThreadPoolExecutor(2),
    }
    _nc_cache["rt"] = rt
    return rt


def kernel(s, nrows, ncols):
    s = np.ascontiguousarray(np.asarray(s, dtype=np.float32))
    nrows = np.asarray(nrows, dtype=np.int32)
    ncols = np.asarray(ncols, dtype=np.int32)

    rt = _get_runtime()
    jax = rt["jax"]

    nrc = np.stack([nrows, ncols], axis=1).astype(np.float32)

    # async uploads: tiny nrc first, then the single big s put
    dev_nrc = jax.device_put(nrc, rt["sh"])
    dev_s = jax.device_put(s, rt["sh"])

    ins = {"s": dev_s, "nrc": dev_nrc}
    outs = rt["sharded"](*[ins[n] for n in rt["in_names"]])
    omap = dict(zip(rt["out_names"], outs))

    # host prep while the upload streams
    out = np.empty((B, N1, N2), dtype=np.float32)
    act = (np.arange(N1, dtype=np.int64)[None, :] < nrows[:, None]).astype(
        np.float32
    )

    if os.environ.get("LAP_OUT", "u8") == "f32":
        res = np.asarray(omap["out"])
        out[:] = res.reshape(B, N1, N2)
        return out

    sums = np.asarray(omap["sums"]).reshape(B, N1)  # blocks until exec done
    factor = act / (255.0 * np.maximum(sums, 1e-30))  # [B, N1]

    q = omap["qout"]
    pool = rt["pool"]

    def dequant(sl, arr):
        np.multiply(arr, factor[sl][:, :, None], out=out[sl])

    if os.environ.get("LAP_NO_OVERLAP", "0") == "1":
        q_np = np.asarray(q).reshape(B, N1, N2)
        np.multiply(q_np, factor[:, :, None], out=out)
        return out

    prev = None
    for shard in q.addressable_shards:
        sl = shard.index[0]  # slice into the global batch axis
        arr = np.asarray(shard.data)  # fetch this core's 4.2MB
        if prev is not None:
            prev.result()
        prev = pool.submit(dequant, sl, arr)
    if prev is not None:
        prev.result()
    return out
